# revision 36
# baseline (speedup 1.0000x reference)
"""Trainium2 Bass kernel for nn_LiquidGenerator.

score = sum over (i, image j) pairs of (CUTOFF - dist)^2 where dist < CUTOFF,
with dist over the [N, 27N] supercell distance matrix.

Strategy (v3)
-------------
Host (O(N log N) prep):
  * generate P (rotation+translation of molecule-local coords, float64)
  * z-sort atoms; rows are processed as 8 chunks of 128 = consecutive z-slabs.
  * central pair symmetry d(i,j)==d(j,i): for row-chunk r only columns j in
    HIGHER chunks are computed (weight 2) plus the full diagonal block
    (weight 1, both orderings).
  * shift symmetry d(i,(k,j)) == d(j,(26-k,i)): one member of each of the 13
    image pairs is computed with weight 2; WHICH member is chosen greedily to
    flatten the per-chunk column loads (the two choices land on mirrored z
    ranges).
  * z-band pruning: a column (central atom or image at z') only pairs with
    chunk r if [z'-3, z'+3] overlaps the chunk's z-slab (~4x fewer elements).
  * distances via the 5-feature inner product
      d^2 + BIAS = [Px,Py,Pz,|P|^2,1] . [-2Sx,-2Sy,-2Sz, 1, |S|^2+BIAS].

Device (8 NeuronCores; every block's columns are sharded core k <- cols k::8):
  per iteration one 4-bank PSUM tile holds 8 uniform units [diag(16)|w2(WM)],
  two per bank: unit = one chunk's diag + weight-2 columns, one self-loading
  fp32 matmul each (8 matmuls, 4-way row-group concurrency).  The weight-2
  factor is folded into the VALUES, not the accumulation:
      sqrt-w2 pass uses scale=2:  s~ = sqrt(2(d^2+B)) = sqrt2 * s
      v' = min(s~, 3*sqrt2) - 3*sqrt2 = sqrt2 * (min(s,3)-3)
  so v'^2 = 2 v^2 and ONE scalar_tensor_tensor square-accumulate over the
  whole tile yields sum(v_diag^2) + 2 sum(v_w2^2) in a single accumulator
  (one DVE accumulator-read per iteration).  All terms are exactly zero for
  non-contributing pairs: no big-sum cancellation, sqrt-spline-safe.
    ScalarE : s~ = sqrt(2(d^2+B)) over w2, s = sqrt(d^2+B) over diag
    VectorE : v' = min(s,3)-3 / min(s~,3sqrt2)-3sqrt2   (bf16, 4x mode)
    VectorE : acc += v'*v' (scalar_tensor_tensor, 2x mode, accum_out)
  score = sum acc - N (3-sqrt(BIAS))^2 + N (3-sqrt(EPS))^2

The timing loop uses a DYNAMIC trip count (read from the `loopn` input) so
one compiled program serves every loop length: the PJRT dispatch constant
cancels exactly in the (wall(N) - wall(1)) / (N-1) slope.  The body holds
`reps` back-to-back iterations so consecutive ones pipeline through the
double-buffered PSUM/SBUF tiles and the all-engine barrier amortizes.
"""

import numpy as np

CUTOFF = 3.0
EPS = 1e-16
BIAS = 2e-4
BAND_MARGIN = 1e-3

NCORES = 8
N = 1024
NCHUNK = 8
G = 4                # PE row groups == PSUM banks
RHS0 = 256           # rhs feature columns start after the two lhs blocks

_cache: dict = {}


# ----------------------------------------------------------------- host math
def _rotation_matrices(rot):
    a, b, g = rot[:, 0], rot[:, 1], rot[:, 2]
    ca, sa = np.cos(a), np.sin(a)
    cb, sb = np.cos(b), np.sin(b)
    cg, sg = np.cos(g), np.sin(g)
    m = rot.shape[0]
    rx = np.zeros((m, 3, 3)); ry = np.zeros((m, 3, 3)); rz = np.zeros((m, 3, 3))
    rx[:, 0, 0] = 1;  rx[:, 1, 1] = ca; rx[:, 1, 2] = -sa; rx[:, 2, 1] = sa; rx[:, 2, 2] = ca
    ry[:, 0, 0] = cb; ry[:, 0, 2] = -sb; ry[:, 1, 1] = 1;  ry[:, 2, 0] = sb; ry[:, 2, 2] = cb
    rz[:, 0, 0] = cg; rz[:, 0, 1] = -sg; rz[:, 1, 0] = sg; rz[:, 1, 1] = cg; rz[:, 2, 2] = 1
    return np.einsum("mij,mjk,mkl->mil", rx, ry, rz)


def _generate(positions, translation, rotation, cell):
    R = _rotation_matrices(rotation.astype(np.float64))
    trans = np.remainder(translation.astype(np.float64), 1.0) @ cell.astype(np.float64)
    gen = np.einsum("mai,mij->maj", positions.astype(np.float64), R) + trans[:, None, :]
    return gen.reshape(-1, 3)


def _features(S, c, bias):
    """rhs feature columns for image positions S (pairs with lhs features)."""
    Sc = (S - c).astype(np.float32)
    return np.stack([
        -2.0 * Sc[:, 0], -2.0 * Sc[:, 1], -2.0 * Sc[:, 2],
        np.ones(S.shape[0], np.float32),
        (Sc.astype(np.float64) ** 2).sum(1).astype(np.float32) + np.float32(bias),
    ]).astype(np.float32)


def _featT(Patoms, c):
    """lhs feature rows [5, n] for row atoms."""
    Pc = (Patoms - c).astype(np.float32)
    return np.stack([
        Pc[:, 0], Pc[:, 1], Pc[:, 2],
        (Pc.astype(np.float64) ** 2).sum(1).astype(np.float32),
        np.ones(Patoms.shape[0], np.float32),
    ]).astype(np.float32)


# ------------------------------------------------------------- bass program
def _build_program(w2b: int, reps: int = 1, dyn_loop: bool = False,
                   parts: str = "full"):
    # w2b: per-core padded weight-2 width per bank (2 chunks' w2 cols).
    # parts: "full" | "mm" | "mm+act" | "mm+act+ts" | "noaccum"  (bisection)
    key = ("nc", w2b, reps, dyn_loop, parts)
    if key in _cache:
        return _cache[key]
    from contextlib import ExitStack, nullcontext
    import concourse.tile as tile
    from concourse import bacc, mybir

    f32 = mybir.dt.float32
    bf16 = mybir.dt.bfloat16
    i32 = mybir.dt.int32
    BW = 32 + w2b                     # live PSUM cols per bank
    W2A = G * w2b                     # total w2 cols (s-tile region size)
    FD = G * BW                       # elements per partition per iteration
    # lhs: 2 chunks' features K-stacked per row group (K=10, rows 32g..+10);
    # rhs: one [diagA diagB w2AB] block per group; zero rows kill
    # cross-chunk terms.
    FW = 128 + BW + 64
    T2 = float(np.float32(3.0 * np.sqrt(2.0)))

    nc = bacc.Bacc("TRN2", target_bir_lowering=False, debug=False,
                   num_devices=NCORES)
    feat_d = nc.dram_tensor("feat", [128, FW], f32, kind="ExternalInput")
    if dyn_loop:
        loopn_d = nc.dram_tensor("loopn", [1, 1], i32, kind="ExternalInput")
    acc_d = nc.dram_tensor("acc", [128, 1], f32, kind="ExternalOutput")

    with tile.TileContext(nc) as tc, ExitStack() as ctx:
        const = ctx.enter_context(tc.tile_pool(name="const", bufs=1))
        psum = ctx.enter_context(tc.tile_pool(name="psum", bufs=2, space="PSUM"))
        spool = ctx.enter_context(tc.tile_pool(name="s", bufs=3))
        vpool = ctx.enter_context(tc.tile_pool(name="v", bufs=3))
        qpool = ctx.enter_context(tc.tile_pool(name="q", bufs=3))

        ft = const.tile([128, FW], f32)
        nc.sync.dma_start(ft[:], feat_d[:])
        at = const.tile([128, 1], f32)
        if parts != "full":
            nc.vector.memset(at[:], 0.0)   # bisection variants never write it

        # bf16-zero views of the zero-padded feat tail for the toucher matmul
        bw = ft[0:1, FW - 64:FW].bitcast(bf16)  # [1,128]
        bx = bw[:, 0:1]

        if dyn_loop:
            lt = const.tile([1, 1], i32)
            nc.sync.dma_start(lt[:], loopn_d[:])
            nval = nc.values_load(lt[0:1, 0:1], min_val=1, max_val=1 << 30,
                                  skip_runtime_bounds_check=True)
            loop_cm = tc.For_i(0, nval, 1)
        else:
            loop_cm = nullcontext()
        with loop_cm:
            for _u in range(reps):
                ps = psum.tile([128, G * 512], f32)
                for g in range(G):
                    # 2 chunks K-stacked (K=10): one matmul covers both
                    # chunks' [diagA diagB w2AB] block; each column's rhs
                    # rows outside its own chunk's 5 features are zero, so
                    # cross-chunk terms vanish exactly.
                    fl = ft[32 * g:32 * g + 10, :]
                    nc.tensor.matmul(
                        ps[:, g * 512:g * 512 + BW],
                        fl[:, 0:128],
                        fl[:, 128:128 + BW],
                        start=True, stop=True, tile_position=(32 * g, 0))

                # s-tile mirrors the live PSUM region: [dA dB w2AB] per bank,
                # 4 banks packed; ONE sqrt covers diag and w2 together (the
                # w2 rhs features are pre-scaled 2x on the host, so PSUM
                # already holds 2(d^2+B) there: s~ = sqrt2 * s, no scale op).
                pb = ps[:].rearrange("p (b w) -> p b w", b=G)[:, :, 0:BW]
                st = spool.tile([128, G * BW], bf16)
                jv = vpool.tile([128, G * BW], bf16)
                jq = qpool.tile([128, G * BW], bf16)
                s3 = st[:].rearrange("p (b w) -> p b w", b=G)
                v3 = jv[:].rearrange("p (b w) -> p b w", b=G)

                if parts != "mm":
                    nc.scalar.activation(s3, pb,
                                         mybir.ActivationFunctionType.Sqrt)
                if parts not in ("mm", "mm+act"):
                    # VectorE: v' = min(s~,3sqrt2)-3sqrt2 / min(s,3)-3 (bf16 4x)
                    nc.vector.tensor_scalar(
                        v3[:, :, 32:BW], s3[:, :, 32:BW], T2, T2,
                        mybir.AluOpType.min, mybir.AluOpType.subtract)
                    nc.vector.tensor_scalar(
                        v3[:, :, 0:32], s3[:, :, 0:32], CUTOFF, CUTOFF,
                        mybir.AluOpType.min, mybir.AluOpType.subtract)
                if parts in ("full", "noaccum"):
                    # VectorE: acc = sum v'^2 (single accumulator, 2x mode)
                    nc.vector.scalar_tensor_tensor(
                        jq[:], jv[:], 1.0, jv[:],
                        mybir.AluOpType.mult, mybir.AluOpType.mult,
                        accum_out=at[:, 0:1] if parts == "full" else None)
                # No PSUM toucher: ACT is the only PSUM reader, so the WAR
                # wait lands on rep u+2's first matmul (a single legal wait)
                # and the PE runs two reps ahead of the ScalarE.
        nc.sync.dma_start(acc_d[:], at[:])

    nc.finalize()
    _cache[key] = nc
    return nc


# --------------------------------------------------------------- input prep
def _prepare_inputs(positions, translation, rotation, cell):
    cell64 = cell.astype(np.float64)
    P = _generate(positions, translation, rotation, cell64)      # [N,3] float64
    n = P.shape[0]
    assert n == N, f"kernel hardcodes N={N}, got {n}"

    order = np.argsort(P[:, 2], kind="stable")
    Ps = P[order]
    zs = Ps[:, 2]
    slab_lo = zs.reshape(NCHUNK, 128).min(1)
    slab_hi = zs.reshape(NCHUNK, 128).max(1)

    shifts = np.array([-1.0, 0.0, 1.0])
    offs = np.stack(np.meshgrid(shifts, shifts, shifts, indexing="ij")).reshape(3, -1).T
    vecs = offs @ cell64                                          # [27,3]
    assert np.all(offs[13] == 0.0)

    c = 0.5 * cell64.sum(axis=0)
    reach = CUTOFF + BAND_MARGIN
    lo = P.min(axis=0) - reach
    hi = P.max(axis=0) + reach

    def chunk_cols(S):
        """per-chunk kept image positions for image set S (z-band + box)."""
        keep = np.all((S > lo) & (S < hi), axis=1)
        out = []
        for r in range(NCHUNK):
            m = keep & (S[:, 2] >= slab_lo[r] - reach) & (S[:, 2] <= slab_hi[r] + reach)
            out.append(S[m])
        return out

    # Two kinds of two-sided choices, greedily assigned to flatten the
    # padded per-bank width = max big-with-small pair sum of chunk loads:
    #  * central chunk-pair (r,q): the weight-2 block can sit at rows r
    #    (cols = q's atoms within reach) or rows q (cols = r's atoms)
    #  * half-shift pair (k, 26-k): mirrored image columns land on
    #    mirrored z ranges
    def pair_cost(ld):
        s = np.sort(ld)
        return int(np.max(s + s[::-1]))

    items = []
    for r in range(NCHUNK):
        for q in range(r + 1, NCHUNK):
            a = Ps[128 * q:128 * (q + 1)]
            a = a[a[:, 2] <= slab_hi[r] + reach]
            b = Ps[128 * r:128 * (r + 1)]
            b = b[b[:, 2] >= slab_lo[q] - reach]
            if len(a) == 0 and len(b) == 0:
                continue
            la = np.zeros(NCHUNK, int); la[r] = len(a)
            lb = np.zeros(NCHUNK, int); lb[q] = len(b)
            items.append((la, {r: a} if len(a) else {},
                          lb, {q: b} if len(b) else {}))
    for k in range(13):
        ca = chunk_cols(Ps + vecs[k])
        cb = chunk_cols(Ps + vecs[26 - k])
        items.append((np.array([len(x) for x in ca]),
                      {r: ca[r] for r in range(NCHUNK) if len(ca[r])},
                      np.array([len(x) for x in cb]),
                      {r: cb[r] for r in range(NCHUNK) if len(cb[r])}))

    perch = [[] for _ in range(NCHUNK)]
    loads = np.zeros(NCHUNK, int)
    items.sort(key=lambda it: -max(it[0].sum(), it[2].sum()))
    for la, da, lb, db in items:
        if pair_cost(loads + la) <= pair_cost(loads + lb):
            lp, dp = la, da
        else:
            lp, dp = lb, db
        loads = loads + lp
        for r, cols in dp.items():
            perch[r].append(cols)

    w2_pos = [np.concatenate(perch[r], axis=0) if perch[r] else np.zeros((0, 3))
              for r in range(NCHUNK)]
    # per-core widths; pair big-with-small chunks per bank to minimize the
    # padded per-bank w2 width
    pw = np.array([-(-len(p) // NCORES) for p in w2_pos])
    idx = np.argsort(pw, kind="stable")[::-1]
    pairs = [(int(idx[b]), int(idx[7 - b])) for b in range(G)]
    w2b = int(max(pw[a] + pw[b] for a, b in pairs))
    w2b = -(-w2b // 4) * 4                                        # multiple of 4
    BW = 32 + w2b

    dummy_pos = c + 50.0                                          # d^2 >> 9

    in_maps = []
    for core in range(NCORES):
        feat = np.zeros((128, 128 + BW + 64), np.float32)
        for g, (ra, rb) in enumerate(pairs):
            # row group g: chunk ra in rows 32g..+5, rb in rows 32g+5..+10;
            # rhs block [diagA(16) diagB(16) w2A w2B pad]; each column only
            # fills its own chunk's 5 feature rows.
            base = 32 * g
            for u, r in enumerate((ra, rb)):
                rows = slice(base + 5 * u, base + 5 * u + 5)
                feat[rows, 0:128] = _featT(Ps[128 * r:128 * (r + 1)], c)
                dcols = Ps[128 * r:128 * (r + 1)][core::NCORES]   # 16 diag
                feat[rows, 128 + 16 * u:128 + 16 * u + 16] = _features(
                    dcols, c, BIAS)
            wa = int(pw[ra])
            sela = w2_pos[ra][core::NCORES]
            selb = w2_pos[rb][core::NCORES]
            pad = w2b - wa - len(selb)
            assert pad >= 0 and len(sela) <= wa
            pada = wa - len(sela)
            if pada:
                sela = np.concatenate(
                    [sela, np.tile(dummy_pos, (pada, 1))], axis=0)
            if pad:
                selb = np.concatenate(
                    [selb, np.tile(dummy_pos, (pad, 1))], axis=0)
            # w2 rhs features pre-scaled 2x (exact): PSUM gets 2(d^2+B)
            feat[slice(base, base + 5), 128 + 32:128 + 32 + wa] = \
                2.0 * _features(sela, c, BIAS)
            feat[slice(base + 5, base + 10), 128 + 32 + wa:128 + BW] = \
                2.0 * _features(selb, c, BIAS)
        in_maps.append({"feat": np.ascontiguousarray(feat)})
    return in_maps, w2b


# ------------------------------------------------------------------- runner
def _get_runner(wm, reps: int = 1, dyn_loop: bool = False, parts: str = "full"):
    """Jit the bass program once; reuse the compiled executable per call."""
    key = ("runner", wm, reps, dyn_loop, parts)
    if key in _cache:
        return _cache[key]
    import jax
    from jax.sharding import Mesh, PartitionSpec
    from jax.experimental.shard_map import shard_map
    from concourse import bass2jax, mybir

    nc = _build_program(wm, reps=reps, dyn_loop=dyn_loop, parts=parts)
    bass2jax.install_neuronx_cc_hook()

    partition_name = (
        nc.partition_id_tensor.name if nc.partition_id_tensor else None
    )
    in_names, out_names, out_avals, zero_outs = [], [], [], []
    for alloc in nc.m.functions[0].allocations:
        if not isinstance(alloc, mybir.MemoryLocationSet):
            continue
        name = alloc.memorylocations[0].name
        if alloc.kind == "ExternalInput":
            if name != partition_name:
                in_names.append(name)
        elif alloc.kind == "ExternalOutput":
            out_names.append(name)
            shape = tuple(alloc.tensor_shape)
            dtype = mybir.dt.np(alloc.dtype)
            out_avals.append(jax.core.ShapedArray(shape, dtype))
            zero_outs.append(np.zeros(shape, dtype))
    n_params = len(in_names)
    all_in_names = in_names + out_names
    if partition_name is not None:
        all_in_names = all_in_names + [partition_name]

    def _body(*args):
        operands = list(args)
        if partition_name is not None:
            operands.append(bass2jax.partition_id_tensor())
        outs = bass2jax._bass_exec_p.bind(
            *operands,
            out_avals=tuple(out_avals),
            in_names=tuple(all_in_names),
            out_names=tuple(out_names),
            lowering_input_output_aliases=(),
            sim_require_finite=True,
            sim_require_nnan=True,
            nc=nc,
        )
        return tuple(outs)

    devices = jax.devices()[:NCORES]
    mesh = Mesh(np.asarray(devices), ("core",))
    n_outs = len(out_names)
    sharded = jax.jit(
        shard_map(
            _body, mesh=mesh,
            in_specs=(PartitionSpec("core"),) * (n_params + n_outs),
            out_specs=(PartitionSpec("core"),) * n_outs,
            check_rep=False,
        ),
        keep_unused=True,
    )
    concat_zeros = [
        np.zeros((NCORES * z.shape[0], *z.shape[1:]), z.dtype) for z in zero_outs
    ]

    def run(in_maps):
        concat_in = [
            np.concatenate([in_maps[cc][name] for cc in range(NCORES)], axis=0)
            for name in in_names
        ]
        out_arrs = sharded(*concat_in, *concat_zeros)
        return [
            {
                name: np.asarray(out_arrs[i]).reshape(NCORES, *out_avals[i].shape)[cc]
                for i, name in enumerate(out_names)
            }
            for cc in range(NCORES)
        ]

    _cache[key] = run
    return run


def kernel(positions, translation, rotation, cell, _reps=1, _loop_n=0,
           _parts="full"):
    in_maps, wm = _prepare_inputs(
        np.asarray(positions), np.asarray(translation),
        np.asarray(rotation), np.asarray(cell),
    )
    dyn = _loop_n > 0
    if dyn:
        for m in in_maps:
            m["loopn"] = np.array([[_loop_n]], np.int32)
    run = _get_runner(wm, reps=_reps, dyn_loop=dyn, parts=_parts)
    results = run(in_maps)
    total = 0.0
    for r in results:
        total += r["acc"].astype(np.float64).sum()
    # swap device self-pair terms for the exact ones
    total -= N * (CUTOFF - np.sqrt(BIAS)) ** 2
    total += N * (CUTOFF - np.sqrt(np.float32(EPS))) ** 2
    return np.float32(total)
